# revision 10
# baseline (speedup 1.0000x reference)
"""Masked multi-head self-attention on 8 Trainium2 NeuronCores.

Problem: B=4, T=1024, C=1024, H=16 heads (D=64), key-padding mask.
Sharding: core c handles batch b=c//2 and heads [8*(c%2), 8*(c%2)+8)
(data parallel on B x tensor parallel on heads). Each core computes its
partial output projection; host sums the two head-half partials per batch
and adds bp.

Padded tokens (~20%) are irrelevant on device: padded queries are fixed up
on the host, padded keys are masked to exp=0.  The host therefore COMPACTS
each batch to its kept rows (zero-padded up to TD=896 slots; a TD=1024
variant is built lazily in the ~1e-9 probability case a batch keeps >896
rows).  Dead slots get a -1e9 key bias so exp=0.

Per-core device algorithm (everything in "transposed" layouts so the
contraction dim always sits on SBUF partitions):
  QT = Wq_c^T x_b^T   [512, TD]  (head dim on partitions)
  KT = Wk_c^T x_b^T   [512, TD]
  V  = x_b Wv_c       [TD, 512]  (tokens on partitions), augmented per head
                                 with ones columns so att@v also yields
                                 softmax sums
  S^T_h = KT_h^T QT_h scaled 1/8, key-pad mask applied as per-partition
          bias (-1e9) inside the ScalarE exp -> expS (bf16)
  y_aug^T_h = V_aug_h^T expS_h   (PSUM accum over key tiles)
  normalize with reciprocal sums (broadcast across partitions via a
  selector matmul), then out_partial = y^T^T Wp_c.

Fully-padded query rows (reference softmaxes an all -1e9 row => uniform
attention over ALL keys) are fixed up on the host:
  out[b, q_pad, :] = (mean_k x[b]) @ Wv @ Wp + bv @ Wp + bp.
"""

import sys

sys.path.insert(0, "/opt/trn_rl_repo")

import math

import ml_dtypes
import numpy as np

import concourse.bass as bass
import concourse.tile as tile
from concourse import mybir
from concourse.bass_utils import run_bass_kernel_spmd

B, T, C, H = 4, 1024, 1024, 16
D = C // H          # 64 head dim
HL = H // 2         # 8 heads per core
CP = HL * D         # 512 per-core projection width
P = 128
KT = C // P         # 8 contraction subtiles of C
MT = CP // P        # 4 m-tiles of QT/KT
TD_MAIN = 896       # compacted token slots (kept rows; P(kept>896) ~ 1e-9)
BF16 = mybir.dt.bfloat16
F32 = mybir.dt.float32

LAST_RESULTS = None  # BassKernelResults of the most recent run (for test.py)


# ---------------------------------------------------------------------------
# Workaround: this walrus build only accepts ONE sync-wait command per
# instruction, but Tile's sem assignment can attach several. Post-pass: move
# extra waits onto fresh same-engine nops inserted just before the carrier.
def _split_multi_waits(nc):
    n = 0
    for f in nc.m.functions:
        for blk in f.blocks:
            newlist, changed = [], False
            for i in blk.instructions:
                si = i.sync_info
                if si is not None and si.on_wait is not None and len(si.on_wait) > 1:
                    w = list(si.on_wait)
                    for ww in w[:-1]:
                        newlist.append(
                            mybir.InstNoOp(
                                name=f"WSPLIT-{n}",
                                engine=i.engine,
                                sync_info=mybir.SyncInfo(on_wait=[ww], on_update=[]),
                            )
                        )
                        n += 1
                    si.on_wait = [w[-1]]
                    changed = True
                newlist.append(i)
            if changed:
                blk.instructions = newlist


# NTFF profiling hook: bass_utils' axon trace path looks for
# antenv.axon_hooks, which this image lacks. Synthesize it and register the
# ctypes-based profiler from trn_agent_boot so BASS_TRACE=1 yields exec times.
def _register_ntff_hook():
    try:
        import antenv.axon_hooks  # noqa: F401
        return
    except ImportError:
        pass
    try:
        import types

        import antenv
        from trn_agent_boot.trn_boot import _ntff_profile_via_ctypes

        mod = types.ModuleType("antenv.axon_hooks")
        _state = {"hook": None}
        mod.set_axon_ntff_profile_hook = lambda h: _state.__setitem__("hook", h)
        mod.get_axon_ntff_profile_hook = lambda: _state["hook"]
        sys.modules["antenv.axon_hooks"] = mod
        antenv.axon_hooks = mod
        so = "/opt/axon/libaxon_pjrt.so"
        import os

        if os.path.exists(so):
            mod.set_axon_ntff_profile_hook(_ntff_profile_via_ctypes(so))
    except Exception:
        pass


_register_ntff_hook()
# ---------------------------------------------------------------------------


def _build_nc(TD):
    KTT = TD // P                      # key tiles
    CH = [(0, 512), (512, TD)]        # free-dim chunks (PSUM-bank aligned)
    CHC = [(0, 512), (512, C)]        # chunks of the C output dim

    nc = bass.Bass()
    xT = nc.dram_tensor("xT", [C, TD], BF16, kind="ExternalInput")
    wq = nc.dram_tensor("wq", [C, CP], BF16, kind="ExternalInput")
    wk = nc.dram_tensor("wk", [C, CP], BF16, kind="ExternalInput")
    wv = nc.dram_tensor("wv", [C, CP], BF16, kind="ExternalInput")
    wp = nc.dram_tensor("wp", [CP, C], BF16, kind="ExternalInput")
    bq = nc.dram_tensor("bq", [P, MT], F32, kind="ExternalInput")
    bk = nc.dram_tensor("bk", [P, MT], F32, kind="ExternalInput")
    bv = nc.dram_tensor("bv", [P, CP], F32, kind="ExternalInput")
    mk = nc.dram_tensor("mk", [P, KTT], F32, kind="ExternalInput")
    ef = nc.dram_tensor("ef", [HL, CP], BF16, kind="ExternalInput")
    out = nc.dram_tensor("out", [TD, C], F32, kind="ExternalOutput")

    EXP = mybir.ActivationFunctionType.Exp
    LN = mybir.ActivationFunctionType.Ln

    with tile.TileContext(nc) as tc:
        with (
            tc.tile_pool(name="consts", bufs=1) as consts,
            tc.tile_pool(name="expp", bufs=28) as expp,
            tc.tile_pool(name="outp", bufs=6) as outp,
            tc.tile_pool(name="ps2", bufs=2, space="PSUM") as ps2,
            tc.tile_pool(name="psy", bufs=2, space="PSUM") as psy,
            tc.tile_pool(name="ps1", bufs=2, space="PSUM") as ps1,
        ):
            # ---- input DMAs: per-kt chunks so compute starts on chunk 0 ----
            xTr = xT.rearrange("(kt p) t -> p kt t", p=P)
            wqr = wq.rearrange("(kt p) n -> p kt n", p=P)
            wkr = wk.rearrange("(kt p) n -> p kt n", p=P)
            wvr = wv.rearrange("(kt p) n -> p kt n", p=P)
            xT_sb = consts.tile([P, KT, TD], BF16)
            wq_sb = consts.tile([P, KT, CP], BF16)
            wk_sb = consts.tile([P, KT, CP], BF16)
            wv_sb = consts.tile([P, KT, CP], BF16)
            # tiny consts first (gpsimd queue)
            bq_sb = consts.tile([P, MT], F32)
            nc.gpsimd.dma_start(bq_sb[:], bq[:])
            bk_sb = consts.tile([P, MT], F32)
            nc.gpsimd.dma_start(bk_sb[:], bk[:])
            bv_sb = consts.tile([P, CP], F32)
            nc.gpsimd.dma_start(bv_sb[:], bv[:])
            mk_sb = consts.tile([P, KTT], F32)
            nc.gpsimd.dma_start(mk_sb[:], mk[:])
            ef_sb = consts.tile([HL, CP], BF16)
            nc.gpsimd.dma_start(ef_sb[:], ef[:])
            # interleave wq/xT chunks so the kt=0 matmul can start early
            for kt in range(KT):
                nc.scalar.dma_start(wq_sb[:, kt, :], wqr[:, kt, :])
                nc.sync.dma_start(xT_sb[:, kt, :], xTr[:, kt, :])
            for kt in range(KT):
                nc.gpsimd.dma_start(wk_sb[:, kt, :], wkr[:, kt, :])
                nc.scalar.dma_start(wv_sb[:, kt, :], wvr[:, kt, :])
            wp_sb = consts.tile([P, MT, C], BF16)
            nc.sync.dma_start(wp_sb[:], wp.rearrange("(s p) n -> p s n", p=P))

            # ---- persistent SBUF tensors ------------------------------------
            # V_aug layout [p, kt, h, m]: even h -> v at m 0:64, ones col at 96;
            # odd h -> ones col at 32, v at m 64:128; other m columns are never
            # read (attV only harvests the v rows + its ones row), so no zeroing.
            V_sb = consts.tile([P, KTT, HL, P], BF16)
            QT_sb = consts.tile([P, MT, TD], BF16)
            KT_sb = consts.tile([P, MT, TD], BF16)
            y_sb = consts.tile([P, MT, TD], BF16)
            # per-head softmax sums staged at (lane 96, block h//2) for even
            # heads and (lane 32, block h//2) for odd heads
            stage = consts.tile([P, MT, TD], BF16)
            sums8 = consts.tile([HL, TD], BF16)
            lns8 = consts.tile([HL, TD], F32)
            rcp8 = consts.tile([HL, TD], BF16)

            V5 = V_sb.rearrange("p kt (hh par) m -> p kt hh par m", par=2)
            nc.vector.memset(V5[:, :, :, 0, 96:97], 1.0)
            nc.vector.memset(V5[:, :, :, 1, 32:33], 1.0)

            def qk_proj(mt):
                for w_sb, b_sb, dst in ((wq_sb, bq_sb, QT_sb), (wk_sb, bk_sb, KT_sb)):
                    pss = [ps1.tile([P, 512], F32, tag="ps1", name=f"qkps{ci}") for ci in range(2)]
                    for kt in range(KT):
                        for ci, (c0, c1) in enumerate(CH):
                            nc.tensor.matmul(
                                pss[ci][:, : c1 - c0],
                                w_sb[:, kt, mt * P : (mt + 1) * P],
                                xT_sb[:, kt, c0:c1],
                                start=(kt == 0),
                                stop=(kt == KT - 1),
                            )
                    for ci, (c0, c1) in enumerate(CH):
                        nc.vector.tensor_scalar_add(
                            dst[:, mt, c0:c1],
                            pss[ci][:, : c1 - c0], b_sb[:, mt : mt + 1],
                        )

            # mt=0 runs during input DMA: interleave Q/K chains per kt so the
            # PE consumes each arriving (wq,wk,xT) chunk without stalling
            psq0 = ps2.tile([P, 1024], F32, tag="ps2", name="psq0")
            psk0 = ps2.tile([P, 1024], F32, tag="ps2", name="psk0")
            for kt in range(KT):
                for c0, c1 in CH:
                    nc.tensor.matmul(
                        psq0[:, c0:c1], wq_sb[:, kt, 0:P], xT_sb[:, kt, c0:c1],
                        start=(kt == 0), stop=(kt == KT - 1),
                    )
                    nc.tensor.matmul(
                        psk0[:, c0:c1], wk_sb[:, kt, 0:P], xT_sb[:, kt, c0:c1],
                        start=(kt == 0), stop=(kt == KT - 1),
                    )
            for c0, c1 in CH:
                nc.vector.tensor_scalar_add(
                    QT_sb[:, 0, c0:c1], psq0[:, c0:c1], bq_sb[:, 0:1]
                )
                nc.vector.tensor_scalar_add(
                    KT_sb[:, 0, c0:c1], psk0[:, c0:c1], bk_sb[:, 0:1]
                )

            # ---- V projection (emitted after pair-0 scores so the PE has
            # work while the wv DMA lands; tokens on partitions) -------------
            def v_proj():
                for tt in range(KTT):
                    psv = ps1.tile([P, 512], F32, tag="ps1")
                    for kt in range(KT):
                        nc.tensor.matmul(
                            psv[:],
                            xT_sb[:, kt, tt * P : (tt + 1) * P],
                            wv_sb[:, kt, :],
                            start=(kt == 0),
                            stop=(kt == KT - 1),
                        )
                    src = psv.rearrange("p (hh par d) -> p par hh d", par=2, d=D)
                    bvr = bv_sb.rearrange("p (hh par d) -> p par hh d", par=2, d=D)
                    nc.vector.tensor_add(V5[:, tt, :, 0, 0:D], src[:, 0], bvr[:, 0])
                    nc.vector.tensor_add(V5[:, tt, :, 1, D:P], src[:, 1], bvr[:, 1])

            # ---- normalize helper (reciprocal sums via ln/exp, broadcast) ---
            def norm_phase(r0, r1, p_list):
                # engine APs must start at partition 0/32/64/96, so each phase
                # recomputes rows [0:r1) (cost is by free size, not rows)
                nc.scalar.activation(lns8[0:r1, :], sums8[0:r1, :], LN)
                nc.scalar.activation(rcp8[0:r1, :], lns8[0:r1, :], EXP, scale=-1.0)
                for p in p_list:
                    bcs = [ps1.tile([P, 512], F32, tag="ps1", name=f"bps{ci}") for ci in range(2)]
                    for ci, (c0, c1) in enumerate(CH):
                        nc.tensor.matmul(
                            bcs[ci][:, : c1 - c0], ef_sb[0:r1, p * P : (p + 1) * P],
                            rcp8[0:r1, c0:c1], start=True, stop=True,
                        )
                    for ci, (c0, c1) in enumerate(CH):
                        nc.vector.tensor_mul(
                            y_sb[:, p, c0:c1], y_sb[:, p, c0:c1], bcs[ci][:, : c1 - c0]
                        )

            # ---- attention per head pair ------------------------------------
            for p in range(MT):
                hA, hB = 2 * p, 2 * p + 1
                eAs, eBs = [], []
                for kt in range(KTT):
                    psA = ps2.tile([P, 1024], F32, tag="ps2", name="psA")
                    psB = ps2.tile([P, 1024], F32, tag="ps2", name="psB")
                    for c0, c1 in CH:
                        nc.tensor.matmul(
                            psA[:, c0:c1],
                            KT_sb[0:D, p, kt * P : (kt + 1) * P],
                            QT_sb[0:D, p, c0:c1],
                            start=True,
                            stop=True,
                        )
                        nc.tensor.matmul(
                            psB[:, c0:c1],
                            KT_sb[D:P, p, kt * P : (kt + 1) * P],
                            QT_sb[D:P, p, c0:c1],
                            start=True,
                            stop=True,
                        )
                    eA = expp.tile([P, TD], BF16, tag="exp")
                    nc.scalar.activation(
                        eA[:], psA[:, :TD], EXP,
                        bias=mk_sb[:, kt : kt + 1], scale=1.0 / math.sqrt(D),
                    )
                    eB = expp.tile([P, TD], BF16, tag="exp")
                    nc.scalar.activation(
                        eB[:], psB[:, :TD], EXP,
                        bias=mk_sb[:, kt : kt + 1], scale=1.0 / math.sqrt(D),
                    )
                    eAs.append(eA)
                    eBs.append(eB)
                if p == 0:
                    v_proj()
                for h, es in ((hA, eAs), (hB, eBs)):
                    yps = [psy.tile([P, 512], F32, tag="psy", name=f"yps{ci}") for ci in range(2)]
                    for kt in range(KTT):
                        for ci, (c0, c1) in enumerate(CH):
                            nc.tensor.matmul(
                                yps[ci][:, : c1 - c0], V_sb[:, kt, h, :],
                                es[kt][:, c0:c1],
                                start=(kt == 0), stop=(kt == KTT - 1),
                            )
                    # harvest: y rows + ones-row softmax sums (lane-aligned)
                    lane = 96 if h % 2 == 0 else 32
                    yr = slice(0, D) if h % 2 == 0 else slice(D, P)
                    for ci, (c0, c1) in enumerate(CH):
                        nc.vector.tensor_copy(y_sb[yr, p, c0:c1], yps[ci][yr, : c1 - c0])
                        nc.scalar.activation(
                            stage[lane : lane + 1, p, c0:c1],
                            yps[ci][lane : lane + 1, : c1 - c0],
                            mybir.ActivationFunctionType.Copy,
                        )
                    nc.sync.dma_start(
                        sums8[h : h + 1, :], stage[lane : lane + 1, p, :]
                    )
                if p + 1 < MT:
                    qk_proj(p + 1)
                if p == 2:
                    norm_phase(0, 6, [0, 1, 2])   # pairs 0-2 while pair 3 runs

            # ---- output projection (prefill s2=0..2 of 2 chains to fill the
            # final-normalize bubble; s2=3 completes after pair 3 normalizes) -
            PRE = 3
            pps = {}

            def out_mm(tt, s2):
                for ci, (c0, c1) in enumerate(CHC):
                    dst = pps[tt]
                    ap = dst[ci][:, : c1 - c0] if isinstance(dst, list) else dst[:, c0:c1]
                    nc.tensor.matmul(
                        ap,
                        y_sb[:, s2, tt * P : (tt + 1) * P],
                        wp_sb[:, s2, c0:c1],
                        start=(s2 == 0), stop=(s2 == MT - 1),
                    )

            def out_drain(tt):
                for ci, (c0, c1) in enumerate(CHC):
                    ot = outp.tile([P, 512], F32, tag="out")
                    dst = pps[tt]
                    ap = dst[ci][:, : c1 - c0] if isinstance(dst, list) else dst[:, c0:c1]
                    if ci == 0:
                        nc.vector.tensor_copy(ot[:], ap)
                    else:
                        nc.scalar.activation(
                            ot[:], ap,
                            mybir.ActivationFunctionType.Copy,
                        )
                    nc.sync.dma_start(out[tt * P : (tt + 1) * P, c0:c1], ot[:])

            for tt in range(PRE):
                if tt < 2:
                    pps[tt] = ps2.tile([P, 1024], F32, tag="ps2", name=f"pj{tt}")
                else:
                    pps[tt] = [
                        psy.tile([P, 512], F32, tag="psy", name=f"pj{tt}c{ci}")
                        for ci in range(2)
                    ]
                for s2 in range(MT - 1):
                    out_mm(tt, s2)

            norm_phase(6, HL, [3])

            for tt in range(PRE):
                out_mm(tt, MT - 1)
                out_drain(tt)
            for tt in range(PRE, KTT):
                pps[tt] = ps2.tile([P, 1024], F32, tag="ps2", name=f"pj{tt}")
                for s2 in range(MT):
                    out_mm(tt, s2)
                out_drain(tt)
    _split_multi_waits(nc)
    return nc


_NCS = {}


def _get_nc(TD):
    if TD not in _NCS:
        _NCS[TD] = _build_nc(TD)
    return _NCS[TD]


def kernel(x, x_padding_judge, Wq, bq, Wk, bk, Wv, bv, Wp, bp):
    global LAST_RESULTS
    x = np.asarray(x, dtype=np.float32)
    pad = np.asarray(x_padding_judge, dtype=np.float32)
    Wq = np.asarray(Wq, dtype=np.float32)
    Wk = np.asarray(Wk, dtype=np.float32)
    Wv = np.asarray(Wv, dtype=np.float32)
    Wp = np.asarray(Wp, dtype=np.float32)
    bq = np.asarray(bq, dtype=np.float32)
    bk = np.asarray(bk, dtype=np.float32)
    bv = np.asarray(bv, dtype=np.float32)
    bp = np.asarray(bp, dtype=np.float32)
    bf = ml_dtypes.bfloat16

    # compact each batch to its kept (unpadded) rows
    kept = [np.nonzero(pad[b] == 0.0)[0] for b in range(B)]
    nks = [k.size for k in kept]
    TD = TD_MAIN if max(nks) <= TD_MAIN else T
    KTT = TD // P

    # selector matrix for broadcasting per-head softmax sums: within each
    # 128-wide m-tile, partitions 0:64 take the even head's sums (staged at
    # lane 96), partitions 64:128 the odd head's (staged at lane 32)
    efm = np.zeros((HL, CP), dtype=np.float32)
    for m in range(CP):
        efm[2 * (m // P) + (m % P) // D, m] = 1.0

    in_maps = []
    for c in range(8):
        b, s = c // 2, c % 2
        cols = slice(s * CP, (s + 1) * CP)
        nk = nks[b]
        xTc = np.zeros((C, TD), dtype=bf)
        xTc[:, :nk] = np.ascontiguousarray(x[b][kept[b]].T).astype(bf)
        mkc = np.full(TD, -1e9, dtype=np.float32)
        mkc[:nk] = 0.0
        in_maps.append({
            "xT": xTc,
            "wq": Wq[:, cols].astype(bf),
            "wk": Wk[:, cols].astype(bf),
            "wv": Wv[:, cols].astype(bf),
            "wp": Wp[cols, :].astype(bf),
            "bq": np.ascontiguousarray(bq[cols].reshape(MT, P).T),
            "bk": np.ascontiguousarray(bk[cols].reshape(MT, P).T),
            "bv": np.broadcast_to(bv[cols], (P, CP)).copy(),
            "mk": np.ascontiguousarray(mkc.reshape(KTT, P).T),
            "ef": efm.astype(bf),
        })

    res = run_bass_kernel_spmd(_get_nc(TD), in_maps, core_ids=list(range(8)))
    LAST_RESULTS = res

    out = np.empty((B, T, C), dtype=np.float32)
    for b in range(B):
        nk = nks[b]
        full = res.results[2 * b]["out"][:nk] + res.results[2 * b + 1]["out"][:nk] + bp
        out[b, kept[b], :] = full

    # fully-padded query rows: uniform attention over ALL keys
    for b in range(B):
        rows = np.nonzero(pad[b] == 1.0)[0]
        if rows.size:
            xbar = x[b].mean(axis=0)
            out[b, rows, :] = (xbar @ Wv + bv) @ Wp + bp
    return out


# revision 11
# speedup vs baseline: 1.0402x; 1.0402x over previous
"""Masked multi-head self-attention on 8 Trainium2 NeuronCores.

Problem: B=4, T=1024, C=1024, H=16 heads (D=64), key-padding mask.
Sharding: core c handles batch b=c//2 and heads [8*(c%2), 8*(c%2)+8)
(data parallel on B x tensor parallel on heads). Each core computes its
partial output projection; host sums the two head-half partials per batch
and adds bp.

Padded tokens (~20%) are irrelevant on device: padded queries are fixed up
on the host, padded keys are masked to exp=0.  The host therefore COMPACTS
each batch to its kept rows (zero-padded up to TD=896 slots; a TD=1024
variant is built lazily in the ~1e-9 probability case a batch keeps >896
rows).  Dead slots get a -1e9 key bias so exp=0.

Per-core device algorithm (everything in "transposed" layouts so the
contraction dim always sits on SBUF partitions):
  QT = Wq_c^T x_b^T   [512, TD]  (head dim on partitions)
  KT = Wk_c^T x_b^T   [512, TD]
  V  = x_b Wv_c       [TD, 512]  (tokens on partitions), augmented per head
                                 with ones columns so att@v also yields
                                 softmax sums
  S^T_h = KT_h^T QT_h scaled 1/8, key-pad mask applied as per-partition
          bias (-1e9) inside the ScalarE exp -> expS (bf16)
  y_aug^T_h = V_aug_h^T expS_h   (PSUM accum over key tiles)
  normalize with reciprocal sums (broadcast across partitions via a
  selector matmul), then out_partial = y^T^T Wp_c.

Fully-padded query rows (reference softmaxes an all -1e9 row => uniform
attention over ALL keys) are fixed up on the host:
  out[b, q_pad, :] = (mean_k x[b]) @ Wv @ Wp + bv @ Wp + bp.
"""

import sys

sys.path.insert(0, "/opt/trn_rl_repo")

import math

import ml_dtypes
import numpy as np

import concourse.bass as bass
import concourse.tile as tile
from concourse import mybir
from concourse.bass_utils import run_bass_kernel_spmd

B, T, C, H = 4, 1024, 1024, 16
D = C // H          # 64 head dim
HL = H // 2         # 8 heads per core
CP = HL * D         # 512 per-core projection width
P = 128
KT = C // P         # 8 contraction subtiles of C
MT = CP // P        # 4 m-tiles of QT/KT
TD_MAIN = 896       # compacted token slots (kept rows; P(kept>896) ~ 1e-9)
BF16 = mybir.dt.bfloat16
F32 = mybir.dt.float32

LAST_RESULTS = None  # BassKernelResults of the most recent run (for test.py)


# ---------------------------------------------------------------------------
# Workaround: this walrus build only accepts ONE sync-wait command per
# instruction, but Tile's sem assignment can attach several. Post-pass: move
# extra waits onto fresh same-engine nops inserted just before the carrier.
def _split_multi_waits(nc):
    n = 0
    for f in nc.m.functions:
        for blk in f.blocks:
            newlist, changed = [], False
            for i in blk.instructions:
                si = i.sync_info
                if si is not None and si.on_wait is not None and len(si.on_wait) > 1:
                    w = list(si.on_wait)
                    for ww in w[:-1]:
                        newlist.append(
                            mybir.InstNoOp(
                                name=f"WSPLIT-{n}",
                                engine=i.engine,
                                sync_info=mybir.SyncInfo(on_wait=[ww], on_update=[]),
                            )
                        )
                        n += 1
                    si.on_wait = [w[-1]]
                    changed = True
                newlist.append(i)
            if changed:
                blk.instructions = newlist


# NTFF profiling hook: bass_utils' axon trace path looks for
# antenv.axon_hooks, which this image lacks. Synthesize it and register the
# ctypes-based profiler from trn_agent_boot so BASS_TRACE=1 yields exec times.
def _register_ntff_hook():
    try:
        import antenv.axon_hooks  # noqa: F401
        return
    except ImportError:
        pass
    try:
        import types

        import antenv
        from trn_agent_boot.trn_boot import _ntff_profile_via_ctypes

        mod = types.ModuleType("antenv.axon_hooks")
        _state = {"hook": None}
        mod.set_axon_ntff_profile_hook = lambda h: _state.__setitem__("hook", h)
        mod.get_axon_ntff_profile_hook = lambda: _state["hook"]
        sys.modules["antenv.axon_hooks"] = mod
        antenv.axon_hooks = mod
        so = "/opt/axon/libaxon_pjrt.so"
        import os

        if os.path.exists(so):
            mod.set_axon_ntff_profile_hook(_ntff_profile_via_ctypes(so))
    except Exception:
        pass


_register_ntff_hook()
# ---------------------------------------------------------------------------


def _build_nc(TD):
    KTT = TD // P                      # key tiles
    CH = [(0, 512), (512, TD)]        # free-dim chunks (PSUM-bank aligned)
    CHC = [(0, 512), (512, C)]        # chunks of the C output dim

    nc = bass.Bass()
    xT = nc.dram_tensor("xT", [C, TD], BF16, kind="ExternalInput")
    wq = nc.dram_tensor("wq", [C, CP], BF16, kind="ExternalInput")
    wk = nc.dram_tensor("wk", [C, CP], BF16, kind="ExternalInput")
    wv = nc.dram_tensor("wv", [C, CP], BF16, kind="ExternalInput")
    wp = nc.dram_tensor("wp", [CP, C], BF16, kind="ExternalInput")
    bq = nc.dram_tensor("bq", [P, MT], F32, kind="ExternalInput")
    bk = nc.dram_tensor("bk", [P, MT], F32, kind="ExternalInput")
    bv = nc.dram_tensor("bv", [P, CP], F32, kind="ExternalInput")
    mk = nc.dram_tensor("mk", [P, KTT], F32, kind="ExternalInput")
    ef = nc.dram_tensor("ef", [HL, CP], BF16, kind="ExternalInput")
    out = nc.dram_tensor("out", [TD, C], F32, kind="ExternalOutput")

    EXP = mybir.ActivationFunctionType.Exp
    LN = mybir.ActivationFunctionType.Ln

    with tile.TileContext(nc) as tc:
        with (
            tc.tile_pool(name="consts", bufs=1) as consts,
            tc.tile_pool(name="expp", bufs=28) as expp,
            tc.tile_pool(name="outp", bufs=6) as outp,
            tc.tile_pool(name="ps2", bufs=2, space="PSUM") as ps2,
            tc.tile_pool(name="psy", bufs=2, space="PSUM") as psy,
            tc.tile_pool(name="ps1", bufs=2, space="PSUM") as ps1,
        ):
            # ---- input DMAs: per-kt chunks so compute starts on chunk 0 ----
            xTr = xT.rearrange("(kt p) t -> p kt t", p=P)
            wqr = wq.rearrange("(kt p) n -> p kt n", p=P)
            wkr = wk.rearrange("(kt p) n -> p kt n", p=P)
            wvr = wv.rearrange("(kt p) n -> p kt n", p=P)
            xT_sb = consts.tile([P, KT, TD], BF16)
            wq_sb = consts.tile([P, KT, CP], BF16)
            wk_sb = consts.tile([P, KT, CP], BF16)
            wv_sb = consts.tile([P, KT, CP], BF16)
            # tiny consts first (gpsimd queue)
            bq_sb = consts.tile([P, MT], F32)
            nc.gpsimd.dma_start(bq_sb[:], bq[:])
            bk_sb = consts.tile([P, MT], F32)
            nc.gpsimd.dma_start(bk_sb[:], bk[:])
            bv_sb = consts.tile([P, CP], F32)
            nc.gpsimd.dma_start(bv_sb[:], bv[:])
            mk_sb = consts.tile([P, KTT], F32)
            nc.gpsimd.dma_start(mk_sb[:], mk[:])
            ef_sb = consts.tile([HL, CP], BF16)
            nc.gpsimd.dma_start(ef_sb[:], ef[:])
            # interleave wq/xT chunks so the kt=0 matmul can start early
            for kt in range(KT):
                nc.scalar.dma_start(wq_sb[:, kt, :], wqr[:, kt, :])
                nc.sync.dma_start(xT_sb[:, kt, :], xTr[:, kt, :])
            for kt in range(KT):
                nc.gpsimd.dma_start(wk_sb[:, kt, :], wkr[:, kt, :])
                nc.scalar.dma_start(wv_sb[:, kt, :], wvr[:, kt, :])
            wp_sb = consts.tile([P, MT, C], BF16)
            nc.sync.dma_start(wp_sb[:], wp.rearrange("(s p) n -> p s n", p=P))

            # ---- persistent SBUF tensors ------------------------------------
            # V_aug layout [p, kt, h, m]: even h -> v at m 0:64, ones col at 96;
            # odd h -> ones col at 32, v at m 64:128; other m columns are never
            # read (attV only harvests the v rows + its ones row), so no zeroing.
            V_sb = consts.tile([P, KTT, HL, P], BF16)
            QT_sb = consts.tile([P, MT, TD], BF16)
            KT_sb = consts.tile([P, MT, TD], BF16)
            y_sb = consts.tile([P, MT, TD], BF16)
            # per-head softmax sums staged at (lane 96, block h//2) for even
            # heads and (lane 32, block h//2) for odd heads
            stage = consts.tile([P, MT, TD], BF16)
            sums8 = consts.tile([HL, TD], BF16)
            lns8 = consts.tile([HL, TD], F32)
            rcp8 = consts.tile([HL, TD], BF16)

            V5 = V_sb.rearrange("p kt (hh par) m -> p kt hh par m", par=2)
            nc.vector.memset(V5[:, :, :, 0, 96:97], 1.0)
            nc.vector.memset(V5[:, :, :, 1, 32:33], 1.0)

            def qk_proj_emitters(mt):
                # flattened qk projection steps, popped between score matmuls
                # to fill ScalarE-exp wait bubbles on the PE
                ems = []
                state = {}
                for wi, (w_sb, b_sb, dst) in enumerate(
                    ((wq_sb, bq_sb, QT_sb), (wk_sb, bk_sb, KT_sb))
                ):
                    for kt in range(KT):
                        def em(wi=wi, w_sb=w_sb, kt=kt):
                            if kt == 0:
                                state[wi] = [
                                    ps1.tile([P, 512], F32, tag="ps1", name=f"qk{wi}{ci}")
                                    for ci in range(2)
                                ]
                            for ci, (c0, c1) in enumerate(CH):
                                nc.tensor.matmul(
                                    state[wi][ci][:, : c1 - c0],
                                    w_sb[:, kt, mt * P : (mt + 1) * P],
                                    xT_sb[:, kt, c0:c1],
                                    start=(kt == 0),
                                    stop=(kt == KT - 1),
                                )
                        ems.append(em)

                    def emb(wi=wi, b_sb=b_sb, dst=dst):
                        for ci, (c0, c1) in enumerate(CH):
                            nc.vector.tensor_scalar_add(
                                dst[:, mt, c0:c1],
                                state[wi][ci][:, : c1 - c0], b_sb[:, mt : mt + 1],
                            )
                    ems.append(emb)
                return ems

            def pop_ems(ems, n):
                for _ in range(n):
                    if ems:
                        ems.pop(0)()

            # mt=0 runs during input DMA: interleave Q/K chains per kt so the
            # PE consumes each arriving (wq,wk,xT) chunk without stalling
            psq0 = ps2.tile([P, 1024], F32, tag="ps2", name="psq0")
            psk0 = ps2.tile([P, 1024], F32, tag="ps2", name="psk0")
            for kt in range(KT):
                for c0, c1 in CH:
                    nc.tensor.matmul(
                        psq0[:, c0:c1], wq_sb[:, kt, 0:P], xT_sb[:, kt, c0:c1],
                        start=(kt == 0), stop=(kt == KT - 1),
                    )
                    nc.tensor.matmul(
                        psk0[:, c0:c1], wk_sb[:, kt, 0:P], xT_sb[:, kt, c0:c1],
                        start=(kt == 0), stop=(kt == KT - 1),
                    )
            for c0, c1 in CH:
                nc.vector.tensor_scalar_add(
                    QT_sb[:, 0, c0:c1], psq0[:, c0:c1], bq_sb[:, 0:1]
                )
                nc.vector.tensor_scalar_add(
                    KT_sb[:, 0, c0:c1], psk0[:, c0:c1], bk_sb[:, 0:1]
                )

            # ---- V projection (emitted after pair-0 scores so the PE has
            # work while the wv DMA lands; tokens on partitions) -------------
            def v_proj():
                for tt in range(KTT):
                    psv = ps1.tile([P, 512], F32, tag="ps1")
                    for kt in range(KT):
                        nc.tensor.matmul(
                            psv[:],
                            xT_sb[:, kt, tt * P : (tt + 1) * P],
                            wv_sb[:, kt, :],
                            start=(kt == 0),
                            stop=(kt == KT - 1),
                        )
                    src = psv.rearrange("p (hh par d) -> p par hh d", par=2, d=D)
                    bvr = bv_sb.rearrange("p (hh par d) -> p par hh d", par=2, d=D)
                    nc.vector.tensor_add(V5[:, tt, :, 0, 0:D], src[:, 0], bvr[:, 0])
                    nc.vector.tensor_add(V5[:, tt, :, 1, D:P], src[:, 1], bvr[:, 1])

            # ---- normalize helper (reciprocal sums via ln/exp, broadcast) ---
            def norm_phase(r0, r1, p_list):
                # engine APs must start at partition 0/32/64/96, so each phase
                # recomputes rows [0:r1) (cost is by free size, not rows)
                nc.scalar.activation(lns8[0:r1, :], sums8[0:r1, :], LN)
                nc.scalar.activation(rcp8[0:r1, :], lns8[0:r1, :], EXP, scale=-1.0)
                for p in p_list:
                    bcs = [ps1.tile([P, 512], F32, tag="ps1", name=f"bps{ci}") for ci in range(2)]
                    for ci, (c0, c1) in enumerate(CH):
                        nc.tensor.matmul(
                            bcs[ci][:, : c1 - c0], ef_sb[0:r1, p * P : (p + 1) * P],
                            rcp8[0:r1, c0:c1], start=True, stop=True,
                        )
                    for ci, (c0, c1) in enumerate(CH):
                        nc.vector.tensor_mul(
                            y_sb[:, p, c0:c1], y_sb[:, p, c0:c1], bcs[ci][:, : c1 - c0]
                        )

            # ---- attention per head pair ------------------------------------
            for p in range(MT):
                hA, hB = 2 * p, 2 * p + 1
                ems = qk_proj_emitters(p + 1) if p + 1 < MT else []
                eAs, eBs = [], []
                for kt in range(KTT):
                    psA = ps2.tile([P, 1024], F32, tag="ps2", name="psA")
                    psB = ps2.tile([P, 1024], F32, tag="ps2", name="psB")
                    for c0, c1 in CH:
                        nc.tensor.matmul(
                            psA[:, c0:c1],
                            KT_sb[0:D, p, kt * P : (kt + 1) * P],
                            QT_sb[0:D, p, c0:c1],
                            start=True,
                            stop=True,
                        )
                        nc.tensor.matmul(
                            psB[:, c0:c1],
                            KT_sb[D:P, p, kt * P : (kt + 1) * P],
                            QT_sb[D:P, p, c0:c1],
                            start=True,
                            stop=True,
                        )
                    eA = expp.tile([P, TD], BF16, tag="exp")
                    nc.scalar.activation(
                        eA[:], psA[:, :TD], EXP,
                        bias=mk_sb[:, kt : kt + 1], scale=1.0 / math.sqrt(D),
                    )
                    eB = expp.tile([P, TD], BF16, tag="exp")
                    nc.scalar.activation(
                        eB[:], psB[:, :TD], EXP,
                        bias=mk_sb[:, kt : kt + 1], scale=1.0 / math.sqrt(D),
                    )
                    eAs.append(eA)
                    eBs.append(eB)
                    pop_ems(ems, 2)
                if p == 0:
                    v_proj()
                for h, es in ((hA, eAs), (hB, eBs)):
                    yps = [psy.tile([P, 512], F32, tag="psy", name=f"yps{ci}") for ci in range(2)]
                    for kt in range(KTT):
                        for ci, (c0, c1) in enumerate(CH):
                            nc.tensor.matmul(
                                yps[ci][:, : c1 - c0], V_sb[:, kt, h, :],
                                es[kt][:, c0:c1],
                                start=(kt == 0), stop=(kt == KTT - 1),
                            )
                    # harvest: y rows + ones-row softmax sums (lane-aligned);
                    # split vector/scalar so the psy bufs free quickly
                    lane = 96 if h % 2 == 0 else 32
                    yr = slice(0, D) if h % 2 == 0 else slice(D, P)
                    for ci, (c0, c1) in enumerate(CH):
                        if ci == 0:
                            nc.vector.tensor_copy(
                                y_sb[yr, p, c0:c1], yps[ci][yr, : c1 - c0]
                            )
                        else:
                            nc.scalar.activation(
                                y_sb[yr, p, c0:c1], yps[ci][yr, : c1 - c0],
                                mybir.ActivationFunctionType.Copy,
                            )
                        nc.scalar.activation(
                            stage[lane : lane + 1, p, c0:c1],
                            yps[ci][lane : lane + 1, : c1 - c0],
                            mybir.ActivationFunctionType.Copy,
                        )
                    nc.sync.dma_start(
                        sums8[h : h + 1, :], stage[lane : lane + 1, p, :]
                    )
                    pop_ems(ems, 2)
                pop_ems(ems, len(ems))
                if p == 2:
                    norm_phase(0, 6, [0, 1, 2])   # pairs 0-2 while pair 3 runs

            # ---- output projection (prefill s2=0..2 of 2 chains to fill the
            # final-normalize bubble; s2=3 completes after pair 3 normalizes) -
            PRE = 3
            pps = {}

            def out_mm(tt, s2):
                for ci, (c0, c1) in enumerate(CHC):
                    dst = pps[tt]
                    ap = dst[ci][:, : c1 - c0] if isinstance(dst, list) else dst[:, c0:c1]
                    nc.tensor.matmul(
                        ap,
                        y_sb[:, s2, tt * P : (tt + 1) * P],
                        wp_sb[:, s2, c0:c1],
                        start=(s2 == 0), stop=(s2 == MT - 1),
                    )

            def out_drain(tt):
                for ci, (c0, c1) in enumerate(CHC):
                    ot = outp.tile([P, 512], F32, tag="out")
                    dst = pps[tt]
                    ap = dst[ci][:, : c1 - c0] if isinstance(dst, list) else dst[:, c0:c1]
                    if ci == 0:
                        nc.vector.tensor_copy(ot[:], ap)
                    else:
                        nc.scalar.activation(
                            ot[:], ap,
                            mybir.ActivationFunctionType.Copy,
                        )
                    nc.sync.dma_start(out[tt * P : (tt + 1) * P, c0:c1], ot[:])

            for tt in range(PRE):
                if tt < 2:
                    pps[tt] = ps2.tile([P, 1024], F32, tag="ps2", name=f"pj{tt}")
                else:
                    pps[tt] = [
                        psy.tile([P, 512], F32, tag="psy", name=f"pj{tt}c{ci}")
                        for ci in range(2)
                    ]
                for s2 in range(MT - 1):
                    out_mm(tt, s2)

            norm_phase(6, HL, [3])

            for tt in range(PRE):
                out_mm(tt, MT - 1)
                out_drain(tt)
            for tt in range(PRE, KTT):
                pps[tt] = ps2.tile([P, 1024], F32, tag="ps2", name=f"pj{tt}")
                for s2 in range(MT):
                    out_mm(tt, s2)
                out_drain(tt)
    _split_multi_waits(nc)
    return nc


_NCS = {}


def _get_nc(TD):
    if TD not in _NCS:
        _NCS[TD] = _build_nc(TD)
    return _NCS[TD]


def kernel(x, x_padding_judge, Wq, bq, Wk, bk, Wv, bv, Wp, bp):
    global LAST_RESULTS
    x = np.asarray(x, dtype=np.float32)
    pad = np.asarray(x_padding_judge, dtype=np.float32)
    Wq = np.asarray(Wq, dtype=np.float32)
    Wk = np.asarray(Wk, dtype=np.float32)
    Wv = np.asarray(Wv, dtype=np.float32)
    Wp = np.asarray(Wp, dtype=np.float32)
    bq = np.asarray(bq, dtype=np.float32)
    bk = np.asarray(bk, dtype=np.float32)
    bv = np.asarray(bv, dtype=np.float32)
    bp = np.asarray(bp, dtype=np.float32)
    bf = ml_dtypes.bfloat16

    # compact each batch to its kept (unpadded) rows
    kept = [np.nonzero(pad[b] == 0.0)[0] for b in range(B)]
    nks = [k.size for k in kept]
    TD = TD_MAIN if max(nks) <= TD_MAIN else T
    KTT = TD // P

    # selector matrix for broadcasting per-head softmax sums: within each
    # 128-wide m-tile, partitions 0:64 take the even head's sums (staged at
    # lane 96), partitions 64:128 the odd head's (staged at lane 32)
    efm = np.zeros((HL, CP), dtype=np.float32)
    for m in range(CP):
        efm[2 * (m // P) + (m % P) // D, m] = 1.0

    in_maps = []
    for c in range(8):
        b, s = c // 2, c % 2
        cols = slice(s * CP, (s + 1) * CP)
        nk = nks[b]
        xTc = np.zeros((C, TD), dtype=bf)
        xTc[:, :nk] = np.ascontiguousarray(x[b][kept[b]].T).astype(bf)
        mkc = np.full(TD, -1e9, dtype=np.float32)
        mkc[:nk] = 0.0
        in_maps.append({
            "xT": xTc,
            "wq": Wq[:, cols].astype(bf),
            "wk": Wk[:, cols].astype(bf),
            "wv": Wv[:, cols].astype(bf),
            "wp": Wp[cols, :].astype(bf),
            "bq": np.ascontiguousarray(bq[cols].reshape(MT, P).T),
            "bk": np.ascontiguousarray(bk[cols].reshape(MT, P).T),
            "bv": np.broadcast_to(bv[cols], (P, CP)).copy(),
            "mk": np.ascontiguousarray(mkc.reshape(KTT, P).T),
            "ef": efm.astype(bf),
        })

    res = run_bass_kernel_spmd(_get_nc(TD), in_maps, core_ids=list(range(8)))
    LAST_RESULTS = res

    out = np.empty((B, T, C), dtype=np.float32)
    for b in range(B):
        nk = nks[b]
        full = res.results[2 * b]["out"][:nk] + res.results[2 * b + 1]["out"][:nk] + bp
        out[b, kept[b], :] = full

    # fully-padded query rows: uniform attention over ALL keys
    for b in range(B):
        rows = np.nonzero(pad[b] == 1.0)[0]
        if rows.size:
            xbar = x[b].mean(axis=0)
            out[b, rows, :] = (xbar @ Wv + bv) @ Wp + bp
    return out


# revision 12
# speedup vs baseline: 1.0415x; 1.0012x over previous
"""Masked multi-head self-attention on 8 Trainium2 NeuronCores.

Problem: B=4, T=1024, C=1024, H=16 heads (D=64), key-padding mask.
Sharding: core c handles batch b=c//2 and heads [8*(c%2), 8*(c%2)+8)
(data parallel on B x tensor parallel on heads). Each core computes its
partial output projection; host sums the two head-half partials per batch
and adds bp.

Padded tokens (~20%) are irrelevant on device: padded queries are fixed up
on the host, padded keys are masked to exp=0.  The host therefore COMPACTS
each batch to its kept rows (zero-padded up to TD=896 slots; a TD=1024
variant is built lazily in the ~1e-9 probability case a batch keeps >896
rows).  Dead slots get a -1e9 key bias so exp=0.

Per-core device algorithm (everything in "transposed" layouts so the
contraction dim always sits on SBUF partitions):
  QT = Wq_c^T x_b^T   [512, TD]  (head dim on partitions)
  KT = Wk_c^T x_b^T   [512, TD]
  V  = x_b Wv_c       [TD, 512]  (tokens on partitions), augmented per head
                                 with ones columns so att@v also yields
                                 softmax sums
  S^T_h = KT_h^T QT_h scaled 1/8, key-pad mask applied as per-partition
          bias (-1e9) inside the ScalarE exp -> expS (bf16)
  y_aug^T_h = V_aug_h^T expS_h   (PSUM accum over key tiles)
  normalize with reciprocal sums (broadcast across partitions via a
  selector matmul), then out_partial = y^T^T Wp_c.

Fully-padded query rows (reference softmaxes an all -1e9 row => uniform
attention over ALL keys) are fixed up on the host:
  out[b, q_pad, :] = (mean_k x[b]) @ Wv @ Wp + bv @ Wp + bp.
"""

import sys

sys.path.insert(0, "/opt/trn_rl_repo")

import math

import ml_dtypes
import numpy as np

import concourse.bass as bass
import concourse.tile as tile
from concourse import mybir
from concourse.bass_utils import run_bass_kernel_spmd

B, T, C, H = 4, 1024, 1024, 16
D = C // H          # 64 head dim
HL = H // 2         # 8 heads per core
CP = HL * D         # 512 per-core projection width
P = 128
KT = C // P         # 8 contraction subtiles of C
MT = CP // P        # 4 m-tiles of QT/KT
TD_MAIN = 896       # compacted token slots (kept rows; P(kept>896) ~ 1e-9)
BF16 = mybir.dt.bfloat16
F32 = mybir.dt.float32

LAST_RESULTS = None  # BassKernelResults of the most recent run (for test.py)


# ---------------------------------------------------------------------------
# Workaround: this walrus build only accepts ONE sync-wait command per
# instruction, but Tile's sem assignment can attach several. Post-pass: move
# extra waits onto fresh same-engine nops inserted just before the carrier.
def _split_multi_waits(nc):
    n = 0
    for f in nc.m.functions:
        for blk in f.blocks:
            newlist, changed = [], False
            for i in blk.instructions:
                si = i.sync_info
                if si is not None and si.on_wait is not None and len(si.on_wait) > 1:
                    w = list(si.on_wait)
                    for ww in w[:-1]:
                        newlist.append(
                            mybir.InstNoOp(
                                name=f"WSPLIT-{n}",
                                engine=i.engine,
                                sync_info=mybir.SyncInfo(on_wait=[ww], on_update=[]),
                            )
                        )
                        n += 1
                    si.on_wait = [w[-1]]
                    changed = True
                newlist.append(i)
            if changed:
                blk.instructions = newlist


# NTFF profiling hook: bass_utils' axon trace path looks for
# antenv.axon_hooks, which this image lacks. Synthesize it and register the
# ctypes-based profiler from trn_agent_boot so BASS_TRACE=1 yields exec times.
def _register_ntff_hook():
    try:
        import antenv.axon_hooks  # noqa: F401
        return
    except ImportError:
        pass
    try:
        import types

        import antenv
        from trn_agent_boot.trn_boot import _ntff_profile_via_ctypes

        mod = types.ModuleType("antenv.axon_hooks")
        _state = {"hook": None}
        mod.set_axon_ntff_profile_hook = lambda h: _state.__setitem__("hook", h)
        mod.get_axon_ntff_profile_hook = lambda: _state["hook"]
        sys.modules["antenv.axon_hooks"] = mod
        antenv.axon_hooks = mod
        so = "/opt/axon/libaxon_pjrt.so"
        import os

        if os.path.exists(so):
            mod.set_axon_ntff_profile_hook(_ntff_profile_via_ctypes(so))
    except Exception:
        pass


_register_ntff_hook()
# ---------------------------------------------------------------------------


def _build_nc(TD):
    KTT = TD // P                      # key tiles
    CH = [(0, 512), (512, TD)]        # free-dim chunks (PSUM-bank aligned)
    CHC = [(0, 512), (512, C)]        # chunks of the C output dim

    nc = bass.Bass()
    xT = nc.dram_tensor("xT", [C, TD], BF16, kind="ExternalInput")
    wq = nc.dram_tensor("wq", [C, CP], BF16, kind="ExternalInput")
    wk = nc.dram_tensor("wk", [C, CP], BF16, kind="ExternalInput")
    wv = nc.dram_tensor("wv", [C, CP], BF16, kind="ExternalInput")
    wp = nc.dram_tensor("wp", [CP, C], BF16, kind="ExternalInput")
    bq = nc.dram_tensor("bq", [P, MT], F32, kind="ExternalInput")
    bk = nc.dram_tensor("bk", [P, MT], F32, kind="ExternalInput")
    bv = nc.dram_tensor("bv", [P, CP], F32, kind="ExternalInput")
    mk = nc.dram_tensor("mk", [P, KTT], F32, kind="ExternalInput")
    ef = nc.dram_tensor("ef", [HL, CP], BF16, kind="ExternalInput")
    out = nc.dram_tensor("out", [TD, C], BF16, kind="ExternalOutput")

    EXP = mybir.ActivationFunctionType.Exp
    LN = mybir.ActivationFunctionType.Ln

    with tile.TileContext(nc) as tc:
        with (
            tc.tile_pool(name="consts", bufs=1) as consts,
            tc.tile_pool(name="expp", bufs=28) as expp,
            tc.tile_pool(name="outp", bufs=6) as outp,
            tc.tile_pool(name="ps2", bufs=2, space="PSUM") as ps2,
            tc.tile_pool(name="psy", bufs=2, space="PSUM") as psy,
            tc.tile_pool(name="ps1", bufs=2, space="PSUM") as ps1,
        ):
            # ---- input DMAs: per-kt chunks so compute starts on chunk 0 ----
            xTr = xT.rearrange("(kt p) t -> p kt t", p=P)
            wqr = wq.rearrange("(kt p) n -> p kt n", p=P)
            wkr = wk.rearrange("(kt p) n -> p kt n", p=P)
            wvr = wv.rearrange("(kt p) n -> p kt n", p=P)
            xT_sb = consts.tile([P, KT, TD], BF16)
            wq_sb = consts.tile([P, KT, CP], BF16)
            wk_sb = consts.tile([P, KT, CP], BF16)
            wv_sb = consts.tile([P, KT, CP], BF16)
            # tiny consts first (gpsimd queue)
            bq_sb = consts.tile([P, MT], F32)
            nc.gpsimd.dma_start(bq_sb[:], bq[:])
            bk_sb = consts.tile([P, MT], F32)
            nc.gpsimd.dma_start(bk_sb[:], bk[:])
            bv_sb = consts.tile([P, CP], F32)
            nc.gpsimd.dma_start(bv_sb[:], bv[:])
            mk_sb = consts.tile([P, KTT], F32)
            nc.gpsimd.dma_start(mk_sb[:], mk[:])
            ef_sb = consts.tile([HL, CP], BF16)
            nc.gpsimd.dma_start(ef_sb[:], ef[:])
            # interleave wq/xT chunks so the kt=0 matmul can start early
            for kt in range(KT):
                nc.scalar.dma_start(wq_sb[:, kt, :], wqr[:, kt, :])
                nc.sync.dma_start(xT_sb[:, kt, :], xTr[:, kt, :])
            for kt in range(KT):
                nc.gpsimd.dma_start(wk_sb[:, kt, :], wkr[:, kt, :])
                nc.scalar.dma_start(wv_sb[:, kt, :], wvr[:, kt, :])
            wp_sb = consts.tile([P, MT, C], BF16)
            nc.sync.dma_start(wp_sb[:], wp.rearrange("(s p) n -> p s n", p=P))

            # ---- persistent SBUF tensors ------------------------------------
            # V_aug layout [p, kt, h, m]: even h -> v at m 0:64, ones col at 96;
            # odd h -> ones col at 32, v at m 64:128; other m columns are never
            # read (attV only harvests the v rows + its ones row), so no zeroing.
            V_sb = consts.tile([P, KTT, HL, P], BF16)
            QT_sb = consts.tile([P, MT, TD], BF16)
            KT_sb = consts.tile([P, MT, TD], BF16)
            y_sb = consts.tile([P, MT, TD], BF16)
            # per-head softmax sums staged at (lane 96, block h//2) for even
            # heads and (lane 32, block h//2) for odd heads
            stage = consts.tile([P, MT, TD], BF16)
            sums8 = consts.tile([HL, TD], BF16)
            lns8 = consts.tile([HL, TD], F32)
            rcp8 = consts.tile([HL, TD], BF16)

            V5 = V_sb.rearrange("p kt (hh par) m -> p kt hh par m", par=2)
            nc.vector.memset(V5[:, :, :, 0, 96:97], 1.0)
            nc.vector.memset(V5[:, :, :, 1, 32:33], 1.0)

            def qk_proj_emitters(mt):
                # flattened qk projection steps, popped between score matmuls
                # to fill ScalarE-exp wait bubbles on the PE
                ems = []
                state = {}
                for wi, (w_sb, b_sb, dst) in enumerate(
                    ((wq_sb, bq_sb, QT_sb), (wk_sb, bk_sb, KT_sb))
                ):
                    for kt in range(KT):
                        def em(wi=wi, w_sb=w_sb, kt=kt):
                            if kt == 0:
                                state[wi] = [
                                    ps1.tile([P, 512], F32, tag="ps1", name=f"qk{wi}{ci}")
                                    for ci in range(2)
                                ]
                            for ci, (c0, c1) in enumerate(CH):
                                nc.tensor.matmul(
                                    state[wi][ci][:, : c1 - c0],
                                    w_sb[:, kt, mt * P : (mt + 1) * P],
                                    xT_sb[:, kt, c0:c1],
                                    start=(kt == 0),
                                    stop=(kt == KT - 1),
                                )
                        ems.append(em)

                    def emb(wi=wi, b_sb=b_sb, dst=dst):
                        for ci, (c0, c1) in enumerate(CH):
                            nc.vector.tensor_scalar_add(
                                dst[:, mt, c0:c1],
                                state[wi][ci][:, : c1 - c0], b_sb[:, mt : mt + 1],
                            )
                    ems.append(emb)
                return ems

            def pop_ems(ems, n):
                for _ in range(n):
                    if ems:
                        ems.pop(0)()

            # mt=0 runs during input DMA: interleave Q/K chains per kt so the
            # PE consumes each arriving (wq,wk,xT) chunk without stalling
            psq0 = ps2.tile([P, 1024], F32, tag="ps2", name="psq0")
            psk0 = ps2.tile([P, 1024], F32, tag="ps2", name="psk0")
            for kt in range(KT):
                for c0, c1 in CH:
                    nc.tensor.matmul(
                        psq0[:, c0:c1], wq_sb[:, kt, 0:P], xT_sb[:, kt, c0:c1],
                        start=(kt == 0), stop=(kt == KT - 1),
                    )
                    nc.tensor.matmul(
                        psk0[:, c0:c1], wk_sb[:, kt, 0:P], xT_sb[:, kt, c0:c1],
                        start=(kt == 0), stop=(kt == KT - 1),
                    )
            for c0, c1 in CH:
                nc.vector.tensor_scalar_add(
                    QT_sb[:, 0, c0:c1], psq0[:, c0:c1], bq_sb[:, 0:1]
                )
                nc.vector.tensor_scalar_add(
                    KT_sb[:, 0, c0:c1], psk0[:, c0:c1], bk_sb[:, 0:1]
                )

            # ---- V projection (emitted after pair-0 scores so the PE has
            # work while the wv DMA lands; tokens on partitions) -------------
            def v_proj():
                for tt in range(KTT):
                    psv = ps1.tile([P, 512], F32, tag="ps1")
                    for kt in range(KT):
                        nc.tensor.matmul(
                            psv[:],
                            xT_sb[:, kt, tt * P : (tt + 1) * P],
                            wv_sb[:, kt, :],
                            start=(kt == 0),
                            stop=(kt == KT - 1),
                        )
                    src = psv.rearrange("p (hh par d) -> p par hh d", par=2, d=D)
                    bvr = bv_sb.rearrange("p (hh par d) -> p par hh d", par=2, d=D)
                    nc.vector.tensor_add(V5[:, tt, :, 0, 0:D], src[:, 0], bvr[:, 0])
                    nc.vector.tensor_add(V5[:, tt, :, 1, D:P], src[:, 1], bvr[:, 1])

            # ---- normalize helper (reciprocal sums via ln/exp, broadcast) ---
            def norm_phase(r0, r1, p_list):
                # engine APs must start at partition 0/32/64/96, so each phase
                # recomputes rows [0:r1) (cost is by free size, not rows)
                nc.scalar.activation(lns8[0:r1, :], sums8[0:r1, :], LN)
                nc.scalar.activation(rcp8[0:r1, :], lns8[0:r1, :], EXP, scale=-1.0)
                for p in p_list:
                    bcs = [ps1.tile([P, 512], F32, tag="ps1", name=f"bps{ci}") for ci in range(2)]
                    for ci, (c0, c1) in enumerate(CH):
                        nc.tensor.matmul(
                            bcs[ci][:, : c1 - c0], ef_sb[0:r1, p * P : (p + 1) * P],
                            rcp8[0:r1, c0:c1], start=True, stop=True,
                        )
                    for ci, (c0, c1) in enumerate(CH):
                        nc.vector.tensor_mul(
                            y_sb[:, p, c0:c1], y_sb[:, p, c0:c1], bcs[ci][:, : c1 - c0]
                        )

            # ---- attention per head pair ------------------------------------
            for p in range(MT):
                hA, hB = 2 * p, 2 * p + 1
                ems = qk_proj_emitters(p + 1) if p + 1 < MT else []
                eAs, eBs = [], []
                for kt in range(KTT):
                    psA = ps2.tile([P, 1024], F32, tag="ps2", name="psA")
                    psB = ps2.tile([P, 1024], F32, tag="ps2", name="psB")
                    for c0, c1 in CH:
                        nc.tensor.matmul(
                            psA[:, c0:c1],
                            KT_sb[0:D, p, kt * P : (kt + 1) * P],
                            QT_sb[0:D, p, c0:c1],
                            start=True,
                            stop=True,
                        )
                        nc.tensor.matmul(
                            psB[:, c0:c1],
                            KT_sb[D:P, p, kt * P : (kt + 1) * P],
                            QT_sb[D:P, p, c0:c1],
                            start=True,
                            stop=True,
                        )
                    eA = expp.tile([P, TD], BF16, tag="exp")
                    nc.scalar.activation(
                        eA[:], psA[:, :TD], EXP,
                        bias=mk_sb[:, kt : kt + 1], scale=1.0 / math.sqrt(D),
                    )
                    eB = expp.tile([P, TD], BF16, tag="exp")
                    nc.scalar.activation(
                        eB[:], psB[:, :TD], EXP,
                        bias=mk_sb[:, kt : kt + 1], scale=1.0 / math.sqrt(D),
                    )
                    eAs.append(eA)
                    eBs.append(eB)
                    pop_ems(ems, 2)
                if p == 0:
                    v_proj()
                for h, es in ((hA, eAs), (hB, eBs)):
                    yps = [psy.tile([P, 512], F32, tag="psy", name=f"yps{ci}") for ci in range(2)]
                    for kt in range(KTT):
                        for ci, (c0, c1) in enumerate(CH):
                            nc.tensor.matmul(
                                yps[ci][:, : c1 - c0], V_sb[:, kt, h, :],
                                es[kt][:, c0:c1],
                                start=(kt == 0), stop=(kt == KTT - 1),
                            )
                    # harvest: y rows + ones-row softmax sums (lane-aligned);
                    # split vector/scalar so the psy bufs free quickly
                    lane = 96 if h % 2 == 0 else 32
                    yr = slice(0, D) if h % 2 == 0 else slice(D, P)
                    for ci, (c0, c1) in enumerate(CH):
                        if ci == 0:
                            nc.vector.tensor_copy(
                                y_sb[yr, p, c0:c1], yps[ci][yr, : c1 - c0]
                            )
                        else:
                            nc.scalar.activation(
                                y_sb[yr, p, c0:c1], yps[ci][yr, : c1 - c0],
                                mybir.ActivationFunctionType.Copy,
                            )
                        nc.scalar.activation(
                            stage[lane : lane + 1, p, c0:c1],
                            yps[ci][lane : lane + 1, : c1 - c0],
                            mybir.ActivationFunctionType.Copy,
                        )
                    nc.sync.dma_start(
                        sums8[h : h + 1, :], stage[lane : lane + 1, p, :]
                    )
                    pop_ems(ems, 2)
                    if p == MT - 1 and h == hA:
                        # normalize pairs 0-2 while pair 3's second head runs;
                        # placed here so the broadcast matmuls sit behind
                        # attV-A in the queue (rcp8 is ready by then)
                        norm_phase(0, 6, [0, 1, 2])
                pop_ems(ems, len(ems))

            # ---- output projection (prefill s2=0..2 of 2 chains to fill the
            # final-normalize bubble; s2=3 completes after pair 3 normalizes) -
            PRE = 3
            pps = {}

            def out_mm(tt, s2):
                for ci, (c0, c1) in enumerate(CHC):
                    dst = pps[tt]
                    ap = dst[ci][:, : c1 - c0] if isinstance(dst, list) else dst[:, c0:c1]
                    nc.tensor.matmul(
                        ap,
                        y_sb[:, s2, tt * P : (tt + 1) * P],
                        wp_sb[:, s2, c0:c1],
                        start=(s2 == 0), stop=(s2 == MT - 1),
                    )

            def out_drain(tt):
                for ci, (c0, c1) in enumerate(CHC):
                    ot = outp.tile([P, 512], BF16, tag="out")
                    dst = pps[tt]
                    ap = dst[ci][:, : c1 - c0] if isinstance(dst, list) else dst[:, c0:c1]
                    if ci == 0:
                        nc.vector.tensor_copy(ot[:], ap)
                    else:
                        nc.scalar.activation(
                            ot[:], ap,
                            mybir.ActivationFunctionType.Copy,
                        )
                    nc.sync.dma_start(out[tt * P : (tt + 1) * P, c0:c1], ot[:])

            for tt in range(PRE):
                if tt < 2:
                    pps[tt] = ps2.tile([P, 1024], F32, tag="ps2", name=f"pj{tt}")
                else:
                    pps[tt] = [
                        psy.tile([P, 512], F32, tag="psy", name=f"pj{tt}c{ci}")
                        for ci in range(2)
                    ]
                for s2 in range(MT - 1):
                    out_mm(tt, s2)

            norm_phase(6, HL, [3])

            for tt in range(PRE):
                out_mm(tt, MT - 1)
                out_drain(tt)
            for tt in range(PRE, KTT):
                pps[tt] = ps2.tile([P, 1024], F32, tag="ps2", name=f"pj{tt}")
                for s2 in range(MT):
                    out_mm(tt, s2)
                out_drain(tt)
    _split_multi_waits(nc)
    return nc


_NCS = {}


def _get_nc(TD):
    if TD not in _NCS:
        _NCS[TD] = _build_nc(TD)
    return _NCS[TD]


def kernel(x, x_padding_judge, Wq, bq, Wk, bk, Wv, bv, Wp, bp):
    global LAST_RESULTS
    x = np.asarray(x, dtype=np.float32)
    pad = np.asarray(x_padding_judge, dtype=np.float32)
    Wq = np.asarray(Wq, dtype=np.float32)
    Wk = np.asarray(Wk, dtype=np.float32)
    Wv = np.asarray(Wv, dtype=np.float32)
    Wp = np.asarray(Wp, dtype=np.float32)
    bq = np.asarray(bq, dtype=np.float32)
    bk = np.asarray(bk, dtype=np.float32)
    bv = np.asarray(bv, dtype=np.float32)
    bp = np.asarray(bp, dtype=np.float32)
    bf = ml_dtypes.bfloat16

    # compact each batch to its kept (unpadded) rows
    kept = [np.nonzero(pad[b] == 0.0)[0] for b in range(B)]
    nks = [k.size for k in kept]
    TD = TD_MAIN if max(nks) <= TD_MAIN else T
    KTT = TD // P

    # selector matrix for broadcasting per-head softmax sums: within each
    # 128-wide m-tile, partitions 0:64 take the even head's sums (staged at
    # lane 96), partitions 64:128 the odd head's (staged at lane 32)
    efm = np.zeros((HL, CP), dtype=np.float32)
    for m in range(CP):
        efm[2 * (m // P) + (m % P) // D, m] = 1.0

    in_maps = []
    for c in range(8):
        b, s = c // 2, c % 2
        cols = slice(s * CP, (s + 1) * CP)
        nk = nks[b]
        xTc = np.zeros((C, TD), dtype=bf)
        xTc[:, :nk] = np.ascontiguousarray(x[b][kept[b]].T).astype(bf)
        mkc = np.full(TD, -1e9, dtype=np.float32)
        mkc[:nk] = 0.0
        in_maps.append({
            "xT": xTc,
            "wq": Wq[:, cols].astype(bf),
            "wk": Wk[:, cols].astype(bf),
            "wv": Wv[:, cols].astype(bf),
            "wp": Wp[cols, :].astype(bf),
            "bq": np.ascontiguousarray(bq[cols].reshape(MT, P).T),
            "bk": np.ascontiguousarray(bk[cols].reshape(MT, P).T),
            "bv": np.broadcast_to(bv[cols], (P, CP)).copy(),
            "mk": np.ascontiguousarray(mkc.reshape(KTT, P).T),
            "ef": efm.astype(bf),
        })

    res = run_bass_kernel_spmd(_get_nc(TD), in_maps, core_ids=list(range(8)))
    LAST_RESULTS = res

    out = np.empty((B, T, C), dtype=np.float32)
    for b in range(B):
        nk = nks[b]
        full = (
            res.results[2 * b]["out"][:nk].astype(np.float32)
            + res.results[2 * b + 1]["out"][:nk].astype(np.float32)
            + bp
        )
        out[b, kept[b], :] = full

    # fully-padded query rows: uniform attention over ALL keys
    for b in range(B):
        rows = np.nonzero(pad[b] == 1.0)[0]
        if rows.size:
            xbar = x[b].mean(axis=0)
            out[b, rows, :] = (xbar @ Wv + bv) @ Wp + bp
    return out
